# revision 4
# baseline (speedup 1.0000x reference)
"""Trainium2 Bass kernel for nn_Encoder_79843442033106 (retrieval_knn).

abs-trick fp8 candidate-generation + exact host rescore:

Math: with the per-code GEMM fold gamma_m = (c2_m - cn2_m)/2 and the
per-query fold +xs_q, the PSUM value is t = -2<x,c> + gamma + xs and

    max(side0, side1) = |t| - sigma_m/2 + xs_q,   sigma = c2 + cn2.

So ONE Abs-activation per tile replaces the baseline's two ACT drains +
two DVE combines.  Per 512-col tile: PE 8 fp8 DoubleRow matmuls (853ns),
ACT Abs drain (612ns), Pool (gpsimd) per-code sigma/2 subtract (427ns),
DVE grouped max-reduce over W=32 code groups -- every engine
under the 853ns PE cadence, so the kernel is tensor-bound at the fp8
roofline.  Per-query top-8 groups (DVE InstMax) go to the host, which
exactly rescores every code of every near-best group in f64 and emits
the reference argmin + LSB-first bits.  Device score noise is ~1 (std);
the empirical winner-group rank is <=4 of 63 with margin <=3.9 vs
MARGIN=8 and top-8 kept groups.
"""

import numpy as np
import ml_dtypes

import concourse.bass as bass
import concourse.tile as tile
from concourse import bacc, mybir
from concourse.bass_utils import run_bass_kernel_spmd

B = 64
KSLOT = 16
D = 2016
M = 16001
NBITS = 32
BK = B * KSLOT          # 1024 queries
NCORES = 8
MLOC = 2016             # codes per core
KT = 126                # data rows per contraction tile
NK = 16                 # contraction tiles
W = 32                  # code-group width
CHUNKS = (256, 256, 256, 256, 512, 384, 96)  # column chunks (sum = 2016)
NCH = len(CHUNKS)
GOFF = tuple(np.cumsum((0,) + CHUNKS[:-1]) // W)  # group offset per chunk
COFF = tuple(np.cumsum((0,) + CHUNKS[:-1]))       # column offset per chunk
GTOT = 63               # groups per (core, query)
NQT = 8                 # query tiles of 128
F8 = ml_dtypes.float8_e4m3
MARGIN = 8.0            # host shortlist margin over candidate values
SIGC = 672.0            # sigma/2 centering for fp16 ssig

# DMA issue order: interleave query tiles and code chunks ~1:1 by bytes so
# the PE diagonal never starves.  Issues are spread over the SP and Pool
# queues (Activation stays DMA-free: its Abs drains are near-critical).
# piece -> engine: xN[a|b] = query tile (k-half), cN[a|b] = chunk (k-half),
# sg = the whole sigma tensor.
_DMA_ORDER = (
    ("x0a", "sync"), ("c0a", "gpsimd"), ("x0b", "sync"), ("c0b", "gpsimd"),
    ("sg0", "sync"), ("x1", "sync"), ("c1", "gpsimd"), ("x2", "sync"),
    ("c2", "gpsimd"), ("x3", "sync"), ("c3", "sync"), ("sg1", "sync"),
    ("x4", "sync"), ("c4a", "sync"), ("x5", "sync"), ("c4b", "sync"),
    ("x6", "sync"), ("c5", "sync"), ("x7", "sync"), ("c6", "sync"),
)

# transfer-time model (ns) for the static greedy tile scheduler below:
# [128, Xbytes-per-partition] contiguous piece ~ X * 128/16 / 22.5 ns.
_PIECE_NS = {}
for _qt in range(NQT):
    _PIECE_NS[f"x{_qt}"] = int(2048 * 0.3556)
_PIECE_NS["x0a"] = _PIECE_NS["x0b"] = int(1024 * 0.3556)
for _c, _w in enumerate(CHUNKS):
    _PIECE_NS[f"c{_c}"] = int(16 * _w * 0.3556)
    _PIECE_NS[f"c{_c}a"] = _PIECE_NS[f"c{_c}b"] = int(8 * _w * 0.3556)
_PIECE_NS["sg0"] = int(2 * 1024 * 0.3556)
_PIECE_NS["sg1"] = int(2 * 992 * 0.3556)


# processing order of the last query tile's chunks: big chunks early, a
# 256 chunk between them and the tiny final chunk so the end-of-schedule
# post-processing queues stay shallow
_Q7SEQ = (0, 1, 4, 5, 2, 3, 6)


def _tile_order():
    """Greedy availability-order (chunk, qt) schedule under the DMA model."""
    arr = {}
    t = 2000.0
    for piece, _ in _DMA_ORDER:
        t += _PIECE_NS[piece]
        arr[piece] = t + 900.0
    xa = [max(arr.get(f"x{q}", 0), arr.get("x0b", 0) if q == 0 else 0)
          for q in range(NQT)]
    ca = [max(arr.get(f"c{c}", 0), arr.get(f"c{c}a", 0),
              arr.get(f"c{c}b", 0)) for c in range(NCH)]
    ready = {(c, q): max(xa[q], ca[c]) for c in range(NCH)
             for q in range(NQT)}
    dur = {c: CHUNKS[c] * 5 / 3 for c in range(NCH)}
    order = []
    done = set()
    t = 2500.0
    while len(order) < NCH * NQT:
        cand = [cq for cq in ready if cq not in done]
        avail = [cq for cq in cand if ready[cq] <= t]
        if not avail:
            t = min(ready[cq] for cq in cand)
            continue
        # tiny last-chunk tiles run as soon as their DMA lands; otherwise
        # lowest qt first (clears each query's chunks, so late top-8 chains
        # never pile up), then by _Q7SEQ/size
        c, q = min(avail, key=lambda cq: (
            cq[0] != NCH - 1, cq[1],
            _Q7SEQ.index(cq[0]) if cq[1] == NQT - 1 else 0,
            CHUNKS[cq[0]], cq[0]))
        order.append((c, q))
        done.add((c, q))
        t += dur[c] * (2.0 if t < 3000 else 1.0)
    return tuple(order)


_TILE_ORDER = _tile_order()

_compiled = {}


def _build_program() -> bass.Bass:
    f8d = mybir.dt.float8e4
    f16 = mybir.dt.float16
    f32 = mybir.dt.float32
    u16 = mybir.dt.uint16

    nc = bacc.Bacc("TRN2", debug=False, num_devices=NCORES)

    xqt8 = nc.dram_tensor("xqt8", [NQT, 128, NK, 128], f8d,
                          kind="ExternalInput").ap()
    cts = [nc.dram_tensor(f"ct{c}", [128, NK, w], f8d,
                          kind="ExternalInput").ap()
           for c, w in enumerate(CHUNKS)]
    sgd = nc.dram_tensor("sg", [128, D], f16, kind="ExternalInput").ap()
    outv = nc.dram_tensor("outv", [128, NQT * 8], f16,
                          kind="ExternalOutput").ap()
    outi = nc.dram_tensor("outi", [128, NQT * 8], u16,
                          kind="ExternalOutput").ap()

    with tile.TileContext(nc) as tc:
        with (
            tc.tile_pool(name="const", bufs=1) as cpool,
            tc.tile_pool(name="psum", bufs=8, space="PSUM") as ppool,
            tc.tile_pool(name="work", bufs=6) as wpool,
            tc.tile_pool(name="outs", bufs=1) as opool,
        ):
            txq = cpool.tile([128, NQT, NK, 128], f8d, tag="xq")
            tct = [cpool.tile([128, NK, w], f8d, tag=f"ct{c}",
                              name=f"tct{c}")
                   for c, w in enumerate(CHUNKS)]
            tsg = cpool.tile([128, D], f16, tag="sg")
            ts = opool.tile([128, NQT * GTOT], f16, tag="s")
            tov = opool.tile([128, NQT * 8], f16, tag="ov")
            toi = opool.tile([128, NQT * 8], u16, tag="oi")

            for piece, engname in _DMA_ORDER:
                eng = getattr(nc, engname)
                if piece == "x0a":
                    eng.dma_start(txq[:, 0, 0:8], xqt8[0, :, 0:8])
                elif piece == "x0b":
                    eng.dma_start(txq[:, 0, 8:16], xqt8[0, :, 8:16])
                elif piece == "sg0":
                    eng.dma_start(tsg[:, 0:1024], sgd[:, 0:1024])
                elif piece == "sg1":
                    eng.dma_start(tsg[:, 1024:D], sgd[:, 1024:D])
                elif piece.startswith("x"):
                    qt = int(piece[1:])
                    eng.dma_start(txq[:, qt], xqt8[qt])
                else:
                    c = int(piece[1])
                    sub = piece[2:]
                    if sub == "a":
                        eng.dma_start(tct[c][:, 0:8], cts[c][:, 0:8])
                    elif sub == "b":
                        eng.dma_start(tct[c][:, 8:16], cts[c][:, 8:16])
                    else:
                        eng.dma_start(tct[c][:], cts[c][:])

            qt_tiles_done = [0] * NQT
            for (c, qt) in _TILE_ORDER:
                n = CHUNKS[c]
                g = n // W
                ps = ppool.tile([128, 512], f32, tag="ps")
                for kp in range(NK // 2):
                    nc.tensor.matmul(
                        ps[:, 0:n],
                        lhsT=txq[:, qt, 2 * kp:2 * kp + 2, :],
                        rhs=tct[c][:, 2 * kp:2 * kp + 2, 0:n],
                        start=(kp == 0),
                        stop=(kp == NK // 2 - 1),
                        perf_mode=mybir.MatmulPerfMode.DoubleRow,
                    )
                u16t = wpool.tile([128, 512], f16, tag="u")
                b16t = wpool.tile([128, 512], f16, tag="b")
                # ACT: |t| drain PSUM -> SBUF fp16
                nc.scalar.activation(
                    u16t[:, 0:n], ps[:, 0:n],
                    mybir.ActivationFunctionType.Abs,
                )
                # Pool: per-code sigma/2 subtract
                co = COFF[c]
                nc.gpsimd.tensor_tensor(
                    b16t[:, 0:n], in0=u16t[:, 0:n], in1=tsg[:, co:co + n],
                    op=mybir.AluOpType.subtract,
                )
                # DVE: grouped max-reduce (W-wide code groups); wide chunks
                # first max-combine their two 16-wide half-groups at 2x so
                # the 1x reduce reads half the data
                so = qt * GTOT + GOFF[c]
                bv = b16t[:, 0:n].rearrange("p (g w) -> p g w", w=W)
                if n >= 384:
                    h16t = wpool.tile([128, 256], f16, tag="h")
                    hv = h16t[:, 0:n // 2].rearrange(
                        "p (g w) -> p g w", w=W // 2)
                    nc.vector.tensor_tensor(
                        hv, in0=bv[:, :, 0:W // 2], in1=bv[:, :, W // 2:W],
                        op=mybir.AluOpType.max,
                    )
                    rv = hv
                else:
                    rv = bv
                nc.vector.tensor_reduce(
                    ts[:, so:so + g], rv,
                    axis=mybir.AxisListType.X,
                    op=mybir.AluOpType.max,
                )
                qt_tiles_done[qt] += 1
                if qt_tiles_done[qt] == NCH:
                    nc.vector.max(
                        tov[:, qt * 8:(qt + 1) * 8],
                        ts[:, qt * GTOT:(qt + 1) * GTOT],
                    )
                    nc.vector.max_index(
                        toi[:, qt * 8:(qt + 1) * 8],
                        tov[:, qt * 8:(qt + 1) * 8],
                        ts[:, qt * GTOT:(qt + 1) * GTOT],
                    )
                    if qt == NQT - 1:
                        nc.scalar.dma_start(outv[:], tov[:])
                        nc.sync.dma_start(outi[:], toi[:])

    nc.compile()
    return nc


def _f8(a):
    return np.asarray(a, dtype=np.float32).astype(F8)


def _host_prep(x: np.ndarray, data: np.ndarray):
    """Quantize operands and build the fp8 aug-row folds."""
    xq = np.transpose(
        x.reshape(B, 2, 126, KSLOT, 8), (0, 3, 1, 2, 4)
    ).reshape(BK, D)
    c = data.reshape(M, D)

    xq64 = xq.astype(np.float64)
    c64 = c.astype(np.float64)
    c2 = np.einsum("md,md->m", c64, c64)
    cn2 = D - 2.0 * c64.sum(axis=1) + c2
    xs = xq64.sum(axis=1)
    gamma = (c2 - cn2) / 2.0
    sig2 = (c2 + cn2) / 2.0

    # --- x side: [NQT, 128, NK, 128] fp8, rows 0..125 = fp8(-2x) -------
    x8 = _f8(-2.0 * xq.astype(np.float32))          # [BK, D]
    xqt8 = np.zeros((NQT, 128, NK, 128), dtype=F8)
    xv = np.transpose(
        x8.reshape(NQT, 128, NK, KT), (0, 3, 2, 1)
    )  # [qt, 126, k, 128]
    xqt8[:, 0:KT] = xv
    # gamma fold: x rows (126/127, k0) and (126/127, k1) = -32
    xqt8[:, 126, 0, :] = F8(-32.0)
    xqt8[:, 127, 0, :] = F8(-32.0)
    xqt8[:, 126, 1, :] = F8(-32.0)
    xqt8[:, 127, 1, :] = F8(-32.0)
    # xs fold: rows (126/127, k2) + (126, k3) carry xs/32 hi/lo/lo2
    v = xs / 32.0
    xh = _f8(v)
    r = v - xh.astype(np.float64)
    xl = _f8(r)
    r2 = r - xl.astype(np.float64)
    xl2 = _f8(r2)
    for qt in range(NQT):
        sl = slice(qt * 128, (qt + 1) * 128)
        xqt8[qt, 126, 2, :] = xh[sl]
        xqt8[qt, 127, 2, :] = xl[sl]
        xqt8[qt, 126, 3, :] = xl2[sl]

    # --- c side gamma fold pieces: -gamma/32 in 4 fp8 pieces -----------
    gf = -gamma / 32.0
    ghi = _f8(gf)
    gr = gf - ghi.astype(np.float64)
    glo = _f8(gr)
    gr2 = gr - glo.astype(np.float64)
    glo2 = _f8(gr2)
    gr3 = gr2 - glo2.astype(np.float64)
    glo3 = _f8(gr3)

    ssig = (sig2 - SIGC).astype(np.float16)
    c8 = _f8(c)                                     # [M, D] fp8

    in_maps = []
    for core in range(NCORES):
        s = core * MLOC
        nvalid = min(s + MLOC, M) - s
        sg = np.full(MLOC, 30000.0 - SIGC, dtype=np.float16)
        sg[0:nvalid] = ssig[s:s + nvalid]
        im = {
            "xqt8": xqt8,
            "sg": np.ascontiguousarray(
                np.broadcast_to(sg[None, :], (128, MLOC))
            ),
        }
        for ci, wch in enumerate(CHUNKS):
            cs = COFF[ci]
            wv = min(wch, max(0, nvalid - cs))      # valid cols in chunk
            ct = np.zeros((128, NK, wch), dtype=F8)
            if wv > 0:
                blk = c8[s + cs:s + cs + wv]        # [wv, D]
                ct[0:KT, :, 0:wv] = np.transpose(
                    blk.reshape(wv, NK, KT), (2, 1, 0)
                )
                ct[126, 0, 0:wv] = ghi[s + cs:s + cs + wv]
                ct[127, 0, 0:wv] = glo[s + cs:s + cs + wv]
                ct[126, 1, 0:wv] = glo2[s + cs:s + cs + wv]
                ct[127, 1, 0:wv] = glo3[s + cs:s + cs + wv]
                ct[126, 2, 0:wv] = F8(32.0)
                ct[127, 2, 0:wv] = F8(32.0)
                ct[126, 3, 0:wv] = F8(32.0)
            im[f"ct{ci}"] = ct
        in_maps.append(im)
    return in_maps


def _merge(results, x: np.ndarray, data: np.ndarray):
    """Exact f64 rescore of every code in every near-best candidate group;
    reproduces the reference's argmin over [d0, d1] with its tie order."""
    xq = np.transpose(
        x.reshape(B, 2, 126, KSLOT, 8), (0, 3, 1, 2, 4)
    ).reshape(BK, D).astype(np.float64)
    c64 = data.reshape(M, D).astype(np.float64)
    c2 = np.einsum("md,md->m", c64, c64)
    cn2 = D - 2.0 * c64.sum(axis=1) + c2
    xs = xq.sum(axis=1)

    vals = np.stack([
        np.asarray(r["outv"], dtype=np.float32).reshape(128, NQT, 8)
        for r in results
    ])  # [core, p, qt, 8]
    gids = np.stack([
        np.asarray(r["outi"]).astype(np.int64).reshape(128, NQT, 8)
        for r in results
    ])
    vals = np.transpose(vals, (2, 1, 0, 3)).reshape(BK, NCORES * 8)
    gids = np.transpose(gids, (2, 1, 0, 3)).reshape(BK, NCORES * 8)
    cores = np.broadcast_to(
        np.repeat(np.arange(NCORES), 8)[None, :], (BK, NCORES * 8)
    )

    best = vals.max(axis=1, keepdims=True)
    keep = vals >= best - MARGIN

    gm = cores.astype(np.int64) * MLOC + gids * W       # global first code

    out = np.empty(BK, dtype=np.int64)
    qs, ks = np.nonzero(keep)
    base = gm[qs, ks]
    mm = (base[:, None] + np.arange(W)[None, :]).reshape(-1)
    qq = np.repeat(qs, W)
    ok = mm < M
    mm = mm[ok]
    qq = qq[ok]
    dots = np.empty(len(mm), dtype=np.float64)
    CH = 65536
    for i in range(0, len(mm), CH):
        sl = slice(i, i + CH)
        dots[sl] = np.einsum(
            "pd,pd->p", xq[qq[sl]], c64[mm[sl]]
        )
    s0 = c2[mm] - 2.0 * dots                       # d0 - x2
    s1 = cn2[mm] - 2.0 * (xs[qq] - dots)           # d1 - x2
    cand_val = np.concatenate([s0, s1])
    cand_idx = np.concatenate([mm, mm + M])
    cand_q = np.concatenate([qq, qq])
    order = np.lexsort((cand_idx, cand_val, cand_q))
    cq = cand_q[order]
    first = np.unique(cq, return_index=True)[1]
    assert len(first) == BK, "some query lost all candidates"
    out[cq[first]] = cand_idx[order][first]
    return out


def kernel(x: np.ndarray, data: np.ndarray) -> np.ndarray:
    if "nc" not in _compiled:
        _compiled["nc"] = _build_program()
    nc = _compiled["nc"]

    x = np.asarray(x)
    data = np.asarray(data)
    in_maps = _host_prep(x, data)
    res = run_bass_kernel_spmd(nc, in_maps, list(range(NCORES)))
    _compiled["last_result"] = res

    g = _merge(res.results, x, data).astype(np.int32)
    shifts = np.arange(NBITS, dtype=np.int32)
    bits = (g[:, None] >> shifts[None, :]) & 1
    return bits.astype(np.int32).reshape(B, KSLOT * NBITS)


# revision 5
# speedup vs baseline: 1.0132x; 1.0132x over previous
"""Trainium2 Bass kernel for nn_Encoder_79843442033106 (retrieval_knn).

abs-trick fp8 candidate-generation + exact host rescore:

Math: with the per-code GEMM fold gamma_m = (c2_m - cn2_m)/2 and the
per-query fold +xs_q, the PSUM value is t = -2<x,c> + gamma + xs and

    max(side0, side1) = |t| - sigma_m/2 + xs_q,   sigma = c2 + cn2.

So ONE Abs-activation per tile replaces the baseline's two ACT drains +
two DVE combines.  Per 512-col tile: PE 8 fp8 DoubleRow matmuls (853ns),
ACT Abs drain (612ns), Pool (gpsimd) per-code sigma/2 subtract (427ns),
DVE grouped max-reduce over W=32 code groups -- every engine
under the 853ns PE cadence, so the kernel is tensor-bound at the fp8
roofline.  Per-query top-8 groups (DVE InstMax) go to the host, which
exactly rescores every code of every near-best group in f64 and emits
the reference argmin + LSB-first bits.  Device score noise is ~1 (std);
the empirical winner-group rank is <=4 of 63 with margin <=3.9 vs
MARGIN=8 and top-8 kept groups.
"""

import numpy as np
import ml_dtypes

import concourse.bass as bass
import concourse.tile as tile
from concourse import bacc, mybir
from concourse.bass_utils import run_bass_kernel_spmd

B = 64
KSLOT = 16
D = 2016
M = 16001
NBITS = 32
BK = B * KSLOT          # 1024 queries
NCORES = 8
MLOC = 2016             # codes per core
KT = 126                # data rows per contraction tile
NK = 16                 # contraction tiles
W = 32                  # code-group width
CHUNKS = (256, 256, 256, 256, 512, 384, 96)  # column chunks (sum = 2016)
NCH = len(CHUNKS)
GOFF = tuple(np.cumsum((0,) + CHUNKS[:-1]) // W)  # group offset per chunk
COFF = tuple(np.cumsum((0,) + CHUNKS[:-1]))       # column offset per chunk
GTOT = 63               # groups per (core, query)
NQT = 8                 # query tiles of 128
F8 = ml_dtypes.float8_e4m3
MARGIN = 8.0            # host shortlist margin over candidate values
SIGC = 672.0            # sigma/2 centering for fp16 ssig

# DMA issue order: interleave query tiles and code chunks ~1:1 by bytes so
# the PE diagonal never starves.  Issues are spread over the SP and Pool
# queues (Activation stays DMA-free: its Abs drains are near-critical).
# piece -> engine: xN[a|b] = query tile (k-half), cN[a|b] = chunk (k-half),
# sg = the whole sigma tensor.
_DMA_ORDER = (
    ("x0a", "sync"), ("c0a", "gpsimd"), ("x0b", "sync"), ("c0b", "gpsimd"),
    ("sg0", "sync"), ("x1", "sync"), ("c1", "gpsimd"), ("x2", "sync"),
    ("c2", "gpsimd"), ("x3", "sync"), ("c3", "sync"), ("sg1", "sync"),
    ("x4", "sync"), ("c4a", "sync"), ("x5", "sync"), ("c4b", "sync"),
    ("x6", "sync"), ("c5", "sync"), ("x7", "sync"), ("c6", "sync"),
)

# transfer-time model (ns) for the static greedy tile scheduler below:
# [128, Xbytes-per-partition] contiguous piece ~ X * 128/16 / 22.5 ns.
_PIECE_NS = {}
for _qt in range(NQT):
    _PIECE_NS[f"x{_qt}"] = int(2048 * 0.3556)
_PIECE_NS["x0a"] = _PIECE_NS["x0b"] = int(1024 * 0.3556)
for _c, _w in enumerate(CHUNKS):
    _PIECE_NS[f"c{_c}"] = int(16 * _w * 0.3556)
    _PIECE_NS[f"c{_c}a"] = _PIECE_NS[f"c{_c}b"] = int(8 * _w * 0.3556)
_PIECE_NS["sg0"] = int(2 * 1024 * 0.3556)
_PIECE_NS["sg1"] = int(2 * 992 * 0.3556)


# processing order of the last query tile's chunks: big chunks first so
# the schedule ends on small tiles with short post-processing chains
_Q7SEQ = (4, 5, 0, 1, 2, 3, 6)


def _tile_order():
    """Greedy availability-order (chunk, qt) schedule under the DMA model."""
    arr = {}
    t = 2000.0
    for piece, _ in _DMA_ORDER:
        t += _PIECE_NS[piece]
        arr[piece] = t + 900.0
    xa = [max(arr.get(f"x{q}", 0), arr.get("x0b", 0) if q == 0 else 0)
          for q in range(NQT)]
    ca = [max(arr.get(f"c{c}", 0), arr.get(f"c{c}a", 0),
              arr.get(f"c{c}b", 0)) for c in range(NCH)]
    ready = {(c, q): max(xa[q], ca[c]) for c in range(NCH)
             for q in range(NQT)}
    dur = {c: CHUNKS[c] * 5 / 3 for c in range(NCH)}
    order = []
    done = set()
    t = 2500.0
    while len(order) < NCH * NQT:
        cand = [cq for cq in ready if cq not in done]
        avail = [cq for cq in cand if ready[cq] <= t]
        if not avail:
            t = min(ready[cq] for cq in cand)
            continue
        # tiny last-chunk tiles run as soon as their DMA lands; otherwise
        # lowest qt first (clears each query's chunks, so late top-8 chains
        # never pile up), then by _Q7SEQ/size
        c, q = min(avail, key=lambda cq: (
            cq[0] != NCH - 1, cq[1],
            _Q7SEQ.index(cq[0]) if cq[1] == NQT - 1 else 0,
            CHUNKS[cq[0]], cq[0]))
        order.append((c, q))
        done.add((c, q))
        t += dur[c] * (2.0 if t < 3000 else 1.0)
    return tuple(order)


_TILE_ORDER = _tile_order()

_compiled = {}


def _build_program() -> bass.Bass:
    f8d = mybir.dt.float8e4
    f16 = mybir.dt.float16
    f32 = mybir.dt.float32
    u16 = mybir.dt.uint16

    nc = bacc.Bacc("TRN2", debug=False, num_devices=NCORES)

    xqt8 = nc.dram_tensor("xqt8", [NQT, 128, NK, 128], f8d,
                          kind="ExternalInput").ap()
    cts = [nc.dram_tensor(f"ct{c}", [128, NK, w], f8d,
                          kind="ExternalInput").ap()
           for c, w in enumerate(CHUNKS)]
    sgd = nc.dram_tensor("sg", [128, D], f16, kind="ExternalInput").ap()
    tso = nc.dram_tensor("tso", [128, NQT * GTOT], f16,
                         kind="ExternalOutput").ap()

    with tile.TileContext(nc) as tc:
        with (
            tc.tile_pool(name="const", bufs=1) as cpool,
            tc.tile_pool(name="psum", bufs=8, space="PSUM") as ppool,
            tc.tile_pool(name="work", bufs=6) as wpool,
            tc.tile_pool(name="outs", bufs=1) as opool,
        ):
            txq = cpool.tile([128, NQT, NK, 128], f8d, tag="xq")
            tct = [cpool.tile([128, NK, w], f8d, tag=f"ct{c}",
                              name=f"tct{c}")
                   for c, w in enumerate(CHUNKS)]
            tsg = cpool.tile([128, D], f16, tag="sg")
            ts = opool.tile([128, NQT * GTOT], f16, tag="s")

            for piece, engname in _DMA_ORDER:
                eng = getattr(nc, engname)
                if piece == "x0a":
                    eng.dma_start(txq[:, 0, 0:8], xqt8[0, :, 0:8])
                elif piece == "x0b":
                    eng.dma_start(txq[:, 0, 8:16], xqt8[0, :, 8:16])
                elif piece == "sg0":
                    eng.dma_start(tsg[:, 0:1024], sgd[:, 0:1024])
                elif piece == "sg1":
                    eng.dma_start(tsg[:, 1024:D], sgd[:, 1024:D])
                elif piece.startswith("x"):
                    qt = int(piece[1:])
                    eng.dma_start(txq[:, qt], xqt8[qt])
                else:
                    c = int(piece[1])
                    sub = piece[2:]
                    if sub == "a":
                        eng.dma_start(tct[c][:, 0:8], cts[c][:, 0:8])
                    elif sub == "b":
                        eng.dma_start(tct[c][:, 8:16], cts[c][:, 8:16])
                    else:
                        eng.dma_start(tct[c][:], cts[c][:])

            qt_tiles_done = [0] * NQT
            stored_bulk = False
            for (c, qt) in _TILE_ORDER:
                n = CHUNKS[c]
                g = n // W
                ps = ppool.tile([128, 512], f32, tag="ps")
                for kp in range(NK // 2):
                    nc.tensor.matmul(
                        ps[:, 0:n],
                        lhsT=txq[:, qt, 2 * kp:2 * kp + 2, :],
                        rhs=tct[c][:, 2 * kp:2 * kp + 2, 0:n],
                        start=(kp == 0),
                        stop=(kp == NK // 2 - 1),
                        perf_mode=mybir.MatmulPerfMode.DoubleRow,
                    )
                u16t = wpool.tile([128, 512], f16, tag="u")
                b16t = wpool.tile([128, 512], f16, tag="b")
                # ACT: |t| drain PSUM -> SBUF fp16
                nc.scalar.activation(
                    u16t[:, 0:n], ps[:, 0:n],
                    mybir.ActivationFunctionType.Abs,
                )
                # Pool: per-code sigma/2 subtract
                co = COFF[c]
                nc.gpsimd.tensor_tensor(
                    b16t[:, 0:n], in0=u16t[:, 0:n], in1=tsg[:, co:co + n],
                    op=mybir.AluOpType.subtract,
                )
                # DVE: grouped max-reduce (W-wide code groups); wide chunks
                # first max-combine their two 16-wide half-groups at 2x so
                # the 1x reduce reads half the data
                so = qt * GTOT + GOFF[c]
                bv = b16t[:, 0:n].rearrange("p (g w) -> p g w", w=W)
                if n >= 384:
                    h16t = wpool.tile([128, 256], f16, tag="h")
                    hv = h16t[:, 0:n // 2].rearrange(
                        "p (g w) -> p g w", w=W // 2)
                    nc.vector.tensor_tensor(
                        hv, in0=bv[:, :, 0:W // 2], in1=bv[:, :, W // 2:W],
                        op=mybir.AluOpType.max,
                    )
                    rv = hv
                else:
                    rv = bv
                nc.vector.tensor_reduce(
                    ts[:, so:so + g], rv,
                    axis=mybir.AxisListType.X,
                    op=mybir.AluOpType.max,
                )
                qt_tiles_done[qt] += 1
                # raw group values go to the host (it does the top-k +
                # margin selection itself): bulk store once qts 0..6 are
                # done, then a thin final store gated only on qt7's last
                # reduce
                if (not stored_bulk
                        and all(qt_tiles_done[q] == NCH
                                for q in range(NQT - 1))):
                    nc.sync.dma_start(tso[:, 0:(NQT - 1) * GTOT],
                                      ts[:, 0:(NQT - 1) * GTOT])
                    stored_bulk = True
            nc.sync.dma_start(tso[:, (NQT - 1) * GTOT:],
                              ts[:, (NQT - 1) * GTOT:])

    nc.compile()
    return nc


def _f8(a):
    return np.asarray(a, dtype=np.float32).astype(F8)


def _host_prep(x: np.ndarray, data: np.ndarray):
    """Quantize operands and build the fp8 aug-row folds."""
    xq = np.transpose(
        x.reshape(B, 2, 126, KSLOT, 8), (0, 3, 1, 2, 4)
    ).reshape(BK, D)
    c = data.reshape(M, D)

    xq64 = xq.astype(np.float64)
    c64 = c.astype(np.float64)
    c2 = np.einsum("md,md->m", c64, c64)
    cn2 = D - 2.0 * c64.sum(axis=1) + c2
    xs = xq64.sum(axis=1)
    gamma = (c2 - cn2) / 2.0
    sig2 = (c2 + cn2) / 2.0

    # --- x side: [NQT, 128, NK, 128] fp8, rows 0..125 = fp8(-2x) -------
    x8 = _f8(-2.0 * xq.astype(np.float32))          # [BK, D]
    xqt8 = np.zeros((NQT, 128, NK, 128), dtype=F8)
    xv = np.transpose(
        x8.reshape(NQT, 128, NK, KT), (0, 3, 2, 1)
    )  # [qt, 126, k, 128]
    xqt8[:, 0:KT] = xv
    # gamma fold: x rows (126/127, k0) and (126/127, k1) = -32
    xqt8[:, 126, 0, :] = F8(-32.0)
    xqt8[:, 127, 0, :] = F8(-32.0)
    xqt8[:, 126, 1, :] = F8(-32.0)
    xqt8[:, 127, 1, :] = F8(-32.0)
    # xs fold: rows (126/127, k2) + (126, k3) carry xs/32 hi/lo/lo2
    v = xs / 32.0
    xh = _f8(v)
    r = v - xh.astype(np.float64)
    xl = _f8(r)
    r2 = r - xl.astype(np.float64)
    xl2 = _f8(r2)
    for qt in range(NQT):
        sl = slice(qt * 128, (qt + 1) * 128)
        xqt8[qt, 126, 2, :] = xh[sl]
        xqt8[qt, 127, 2, :] = xl[sl]
        xqt8[qt, 126, 3, :] = xl2[sl]

    # --- c side gamma fold pieces: -gamma/32 in 4 fp8 pieces -----------
    gf = -gamma / 32.0
    ghi = _f8(gf)
    gr = gf - ghi.astype(np.float64)
    glo = _f8(gr)
    gr2 = gr - glo.astype(np.float64)
    glo2 = _f8(gr2)
    gr3 = gr2 - glo2.astype(np.float64)
    glo3 = _f8(gr3)

    ssig = (sig2 - SIGC).astype(np.float16)
    c8 = _f8(c)                                     # [M, D] fp8

    in_maps = []
    for core in range(NCORES):
        s = core * MLOC
        nvalid = min(s + MLOC, M) - s
        sg = np.full(MLOC, 30000.0 - SIGC, dtype=np.float16)
        sg[0:nvalid] = ssig[s:s + nvalid]
        im = {
            "xqt8": xqt8,
            "sg": np.ascontiguousarray(
                np.broadcast_to(sg[None, :], (128, MLOC))
            ),
        }
        for ci, wch in enumerate(CHUNKS):
            cs = COFF[ci]
            wv = min(wch, max(0, nvalid - cs))      # valid cols in chunk
            ct = np.zeros((128, NK, wch), dtype=F8)
            if wv > 0:
                blk = c8[s + cs:s + cs + wv]        # [wv, D]
                ct[0:KT, :, 0:wv] = np.transpose(
                    blk.reshape(wv, NK, KT), (2, 1, 0)
                )
                ct[126, 0, 0:wv] = ghi[s + cs:s + cs + wv]
                ct[127, 0, 0:wv] = glo[s + cs:s + cs + wv]
                ct[126, 1, 0:wv] = glo2[s + cs:s + cs + wv]
                ct[127, 1, 0:wv] = glo3[s + cs:s + cs + wv]
                ct[126, 2, 0:wv] = F8(32.0)
                ct[127, 2, 0:wv] = F8(32.0)
                ct[126, 3, 0:wv] = F8(32.0)
            im[f"ct{ci}"] = ct
        in_maps.append(im)
    return in_maps


def _merge(results, x: np.ndarray, data: np.ndarray):
    """Exact f64 rescore of every code in every near-best candidate group;
    reproduces the reference's argmin over [d0, d1] with its tie order."""
    xq = np.transpose(
        x.reshape(B, 2, 126, KSLOT, 8), (0, 3, 1, 2, 4)
    ).reshape(BK, D).astype(np.float64)
    c64 = data.reshape(M, D).astype(np.float64)
    c2 = np.einsum("md,md->m", c64, c64)
    cn2 = D - 2.0 * c64.sum(axis=1) + c2
    xs = xq.sum(axis=1)

    # raw group maxima from every core: [core, p, qt, GTOT]
    vals = np.stack([
        np.asarray(r["tso"], dtype=np.float32).reshape(128, NQT, GTOT)
        for r in results
    ])
    vals = np.transpose(vals, (2, 1, 0, 3)).reshape(BK, NCORES * GTOT)

    best = vals.max(axis=1, keepdims=True)
    keep = vals >= best - MARGIN

    out = np.empty(BK, dtype=np.int64)
    qs, ks = np.nonzero(keep)
    base = (ks // GTOT).astype(np.int64) * MLOC + (ks % GTOT) * W
    mm = (base[:, None] + np.arange(W)[None, :]).reshape(-1)
    qq = np.repeat(qs, W)
    ok = mm < M
    mm = mm[ok]
    qq = qq[ok]
    dots = np.empty(len(mm), dtype=np.float64)
    CH = 65536
    for i in range(0, len(mm), CH):
        sl = slice(i, i + CH)
        dots[sl] = np.einsum(
            "pd,pd->p", xq[qq[sl]], c64[mm[sl]]
        )
    s0 = c2[mm] - 2.0 * dots                       # d0 - x2
    s1 = cn2[mm] - 2.0 * (xs[qq] - dots)           # d1 - x2
    cand_val = np.concatenate([s0, s1])
    cand_idx = np.concatenate([mm, mm + M])
    cand_q = np.concatenate([qq, qq])
    order = np.lexsort((cand_idx, cand_val, cand_q))
    cq = cand_q[order]
    first = np.unique(cq, return_index=True)[1]
    assert len(first) == BK, "some query lost all candidates"
    out[cq[first]] = cand_idx[order][first]
    return out


def kernel(x: np.ndarray, data: np.ndarray) -> np.ndarray:
    if "nc" not in _compiled:
        _compiled["nc"] = _build_program()
    nc = _compiled["nc"]

    x = np.asarray(x)
    data = np.asarray(data)
    in_maps = _host_prep(x, data)
    res = run_bass_kernel_spmd(nc, in_maps, list(range(NCORES)))
    _compiled["last_result"] = res

    g = _merge(res.results, x, data).astype(np.int32)
    shifts = np.arange(NBITS, dtype=np.int32)
    bits = (g[:, None] >> shifts[None, :]) & 1
    return bits.astype(np.int32).reshape(B, KSLOT * NBITS)
